# revision 26
# baseline (speedup 1.0000x reference)
"""Depthwise conv1d (128 channels, 128 taps, SAME) + softplus on 8 TRN2 cores.

Data-parallel over batch (16 -> 2 per core). Per channel the conv is two
banded-Toeplitz matmuls per 128-step output block (W1 on block j, W2 on block
j+1), accumulated in PSUM; softplus = Ln(Exp(y)+1) on the scalar engine (no
softplus table in this build).

All DMA is f16 with flat per-partition-contiguous access patterns on both
sides (the cost of a DMA is charged on its output AP, with a 2x penalty when
the innermost run is <512B, so layouts are chosen on the host):
  - x is converted to f16, padded by 64 on both ends of T, and pre-arranged on
    the host into per-pass slabs [128, (nb+1)*128] (partition = t mod 128).
  - y is written as f16 into a blocked DRAM layout [pass, p, (c j)] and
    rearranged/upcast to [B, T, C] f32 on the host.
  - W1 is f16, group-major, streamed in 16-channel chunks; W2 is built
    on-chip: one PE matmul per shift (stationary identity strip E, moving raw
    kernels) writes each slab to PSUM exactly, and DVE/GPSIMD copy it to SBUF
    f16 (bit-identical to a host-built table). This halves the startup table
    DMA, which would otherwise starve the activation engine.
The first three passes are processed channel-chunk-major along wavefront
diagonals so Exp work overlaps the W1/x streams; first and last passes are
small to shorten fill/drain. A single pre-placed act-table load (set 6:
natural_log_exp_and_others) covers both Exp and Ln (no table swaps).
"""
import numpy as np
import concourse.mybir as mybir
from concourse import bacc
from concourse.tile import TileContext
from concourse.bass_utils import run_bass_kernel_spmd

AF = mybir.ActivationFunctionType
N_CORES = 8
B, T, C, K = 16, 32768, 128, 128
B_LOCAL = B // N_CORES
NBLK = T // 128              # 256 output blocks per batch row
NGRP = 8                     # channel groups
GRP = C // NGRP              # 16 channels per group
N_STAGE_A = 3                # first passes processed chunk-major
ACT_SET_LN_EXP = 6           # natural_log_exp_and_others in act_info.json
TUNE = {"off": [0, 2, 4], "builds_first": True, "ln_splits": 1}


def _pass_plan(b_local=B_LOCAL):
    """[(bb, start_block, nb), ...]; small first and last passes."""
    plan = []
    for bb in range(b_local):
        first, last = bb == 0, bb == b_local - 1
        if first and last:
            sizes = [16, 48, 64, 64, 48, 16]
        elif first:
            sizes = list(TUNE.get("sizes0", [16, 48, 64, 64, 64]))
        elif last:
            sizes = [64] * ((NBLK - 64) // 64) + [48, 16]
        else:
            sizes = [64] * (NBLK // 64)
        s = 0
        for nb in sizes:
            plan.append((bb, s, nb))
            s += nb
        assert s == NBLK
    return plan


def _build_ws(kernels_np: np.ndarray, dtype=np.float16):
    """Toeplitz tables, group-major: [NGRP, 128, 128*GRP] with
    ws[G][p][i*GRP + u] = T[p, i, G*GRP+u]."""
    w = kernels_np[:, 0, :].astype(np.float32)  # [k, c]
    p = np.arange(128)[:, None, None]
    i = np.arange(128)[None, :, None]
    c = np.arange(128)[None, None, :]
    k1 = p - i - 1
    k2 = p + 127 - i
    cb = np.broadcast_to(c, (128, 128, 128))
    T1 = np.where((k1 >= 0) & (k1 < K), w[np.clip(k1, 0, K - 1), cb], 0.0)
    T2 = np.where((k2 >= 0) & (k2 < K), w[np.clip(k2, 0, K - 1), cb], 0.0)

    def gm(t):  # [p, i, c] -> [G, p, i*GRP+u]
        t = t.reshape(128, 128, NGRP, GRP).transpose(2, 0, 1, 3)
        return np.ascontiguousarray(t.reshape(NGRP, 128, 128 * GRP)).astype(dtype)

    return gm(T1), gm(T2)


def _prep_x(x_np: np.ndarray):
    """[Bb, T, C] f32 -> {x<nb>: [n, 128, (nb+1)*128] f16 slabs}."""
    bb_n = x_np.shape[0]
    x16 = x_np.astype(np.float16)
    xpad = np.zeros((bb_n, T + 128, C), np.float16)
    xpad[:, 64:64 + T] = x16
    xj = xpad.reshape(bb_n, (T + 128) // 128, 128, C)    # [b, J, p, c]
    out = {}
    for bb, s, nb in _pass_plan(bb_n):
        slab = xj[bb, s:s + nb + 1]                      # [nb+1, p, c]
        slab = slab.transpose(1, 0, 2).reshape(128, -1)  # [p, (J c)]
        out.setdefault(f"x{nb}", []).append(slab)
    return {k: np.stack(v) for k, v in out.items()}


def _post_y(y_by_size, b_local=B_LOCAL) -> np.ndarray:
    """{y<nb>: [n, 128, C*nb] f16 blocked} -> [Bb, T, C] f32."""
    out = np.empty((b_local, T, C), np.float32)
    idx = {}
    for bb, s, nb in _pass_plan(b_local):
        i = idx.get(nb, 0)
        idx[nb] = i + 1
        blk = np.asarray(y_by_size[f"y{nb}"][i])          # [p, (c j)] f16
        blk = blk.reshape(128, C, nb).transpose(2, 0, 1)  # [j, p, c]
        out[bb, s * 128:(s + nb) * 128] = blk.reshape(nb * 128, C)
    return out


def build_nc(ws1g, ws2g, b_local=B_LOCAL, num_devices=N_CORES):
    f16, f32 = mybir.dt.float16, mybir.dt.float32
    CHUNK = 128 * GRP  # tile columns per ws group chunk
    plan = _pass_plan(b_local)
    n_a = min(N_STAGE_A, len(plan))
    counts = {}
    for _, _, nb in plan:
        counts[nb] = counts.get(nb, 0) + 1

    # raw kernels [k, c] f16, recovered exactly from the i=127 slab of W2
    w16_np = np.stack([ws2g[c // GRP][:, 127 * GRP + c % GRP]
                       for c in range(C)], axis=1).astype(np.float16)
    # identity strip: E[k, 128+k] = 1; W2 slab i is E[:, 255-i:383-i]^T @ w
    E_np = np.zeros((128, 384), np.float16)
    E_np[np.arange(128), np.arange(128) + 128] = 1.0

    nc = bacc.Bacc("TRN2", target_bir_lowering=False, debug=False,
                   num_devices=num_devices)
    xd = {nb: nc.dram_tensor(f"x{nb}", [n, 128, (nb + 1) * 128], f16,
                             kind="ExternalInput")
          for nb, n in counts.items()}
    yd = {nb: nc.dram_tensor(f"y{nb}", [n, 128, C * nb], f16,
                             kind="ExternalOutput")
          for nb, n in counts.items()}
    ws1_d = nc.inline_tensor(np.ascontiguousarray(ws1g), "ws1")
    ws2_d0 = nc.inline_tensor(np.ascontiguousarray(ws2g[0]), "ws2g0")
    e_d = nc.inline_tensor(E_np, "Estrip")
    w16_d = nc.inline_tensor(np.ascontiguousarray(w16_np), "w16")

    with TileContext(nc) as tc:
        with (
            tc.tile_pool(name="wpool", bufs=1) as wpool,
            tc.tile_pool(name="xpool", bufs=3) as xpool,
            tc.tile_pool(name="epool", bufs=3) as epool,
            tc.tile_pool(name="ypool", bufs=2) as ypool,
            tc.tile_pool(name="ppool", bufs=4, space="PSUM") as ppool,
        ):
            # Both Exp and Ln live in act set 6; one load => no table thrash.
            ld = mybir.InstLoadActFuncSet(
                name=nc.get_next_instruction_name(), ins=[], outs=[])
            ld.act_func_set_id = ACT_SET_LN_EXP
            ld.engine = mybir.EngineType.Activation
            nc.scalar.add_instruction(ld)

            ws1 = wpool.tile([128, NGRP * CHUNK], f16, tag="ws1")
            ws2 = wpool.tile([128, NGRP * CHUNK], f16, tag="ws2")
            et_strip = wpool.tile([128, 384], f16, tag="E")
            w16 = wpool.tile([128, 128], f16, tag="w16")

            nth = {}
            tiles = {}

            def issue_load(idx):
                bb, s, nb = plan[idx]
                i = nth.get(nb, 0)
                nth[nb] = i + 1
                xt = xpool.tile([128, 65 * 128], f16, tag="x")
                nc.sync.dma_start(out=xt[:, 0:(nb + 1) * 128], in_=xd[nb][i])
                tiles[idx] = (xt, i)

            def load_w1(G):
                nc.sync.dma_start(out=ws1[:, G * CHUNK:(G + 1) * CHUNK],
                                  in_=ws1_d.ap()[G])

            proc_a = TUNE.get("proc_a", list(range(n_a)))[:n_a]

            # JIT DMA order: E/w16 (for the W2 builds), then W1 chunks and x
            # slabs interleaved roughly by first use.
            nc.sync.dma_start(out=et_strip[:, :], in_=e_d.ap())
            nc.sync.dma_start(out=w16[:, :], in_=w16_d.ap())
            load_w1(0)
            nc.sync.dma_start(out=ws2[:, 0:CHUNK], in_=ws2_d0.ap())
            for tok in TUNE.get("prologue",
                                ["x:0", "x:1", "w1:1", "w1:2", "x:2", "w1:3",
                                 "w1:4", "w1:5", "w1:6", "w1:7"]):
                kind, _, num = tok.partition(":")
                if kind == "x":
                    issue_load(proc_a[int(num)])
                else:
                    load_w1(int(num))

            ws1v = ws1.rearrange("p (G i u) -> p G i u", G=NGRP, u=GRP)
            ws2v = ws2.rearrange("p (G i u) -> p G i u", G=NGRP, u=GRP)

            def build_w2(G):
                # 128 exact-shift matmuls fill chunk G of W2; two PSUM halves
                # copied to SBUF by DVE and GPSIMD in parallel.
                for half in range(2):
                    ps2 = ppool.tile([128, GRP * 64], f32, tag="ps")
                    for ii in range(64):
                        i = half * 64 + ii
                        nc.tensor.matmul(ps2[:, ii * GRP:(ii + 1) * GRP],
                                         et_strip[:, 255 - i:383 - i],
                                         w16[:, G * GRP:(G + 1) * GRP],
                                         start=True, stop=True)
                    lo = G * CHUNK + half * 64 * GRP
                    nc.vector.tensor_copy(out=ws2[:, lo:lo + 64 * GRP],
                                          in_=ps2[:, :])

            def conv_group(G, idx):
                bb, s, nb = plan[idx]
                xt, _ = tiles[idx]
                x3 = xt.rearrange("p (j c) -> p j c", c=128)
                et, e3 = ets[idx]
                ps = ppool.tile([128, GRP * 64], f32, tag="ps")
                for u in range(GRP):
                    ch = G * GRP + u
                    nc.tensor.matmul(ps[:, u * 64:u * 64 + nb],
                                     ws1v[:, G, :, u], x3[:, 0:nb, ch],
                                     start=True, stop=False)
                    nc.tensor.matmul(ps[:, u * 64:u * 64 + nb],
                                     ws2v[:, G, :, u], x3[:, 1:nb + 1, ch],
                                     start=False, stop=True)
                nc.scalar.activation(
                    e3[:, G * GRP:(G + 1) * GRP, 0:nb],
                    ps.rearrange("p (u j) -> p u j", j=64)[:, :, 0:nb],
                    AF.Exp)

            yts = {}

            def ln_part(idx, c0, c1):
                bb, s, nb = plan[idx]
                et, e3 = ets[idx]
                if idx not in yts:
                    yt_new = ypool.tile([128, C * 64], f16, tag="y")
                    yts[idx] = yt_new
                yt = yts[idx]
                if nb == 64:
                    nc.scalar.activation(yt[:, c0 * 64:c1 * 64],
                                         et[:, c0 * 64:c1 * 64],
                                         AF.Ln, bias=1.0)
                else:
                    y3 = yt[:, c0 * nb:c1 * nb].rearrange(
                        "p (c j) -> p c j", j=nb)
                    nc.scalar.activation(y3[:, :, :], e3[:, c0:c1, 0:nb],
                                         AF.Ln, bias=1.0)

            def do_store(idx):
                bb, s, nb = plan[idx]
                _, yi = tiles.pop(idx)
                ets.pop(idx)
                # last store on the HWDGE path: shorter tail before final drain
                eng = nc.sync if idx == len(plan) - 1 else nc.gpsimd
                yt = yts.pop(idx)
                eng.dma_start(out=yd[nb][yi], in_=yt[:, 0:C * nb])

            def ln_store(idx):
                ln_part(idx, 0, C)
                do_store(idx)

            ets = {}

            def new_et(idx):
                et = epool.tile([128, C * 64], f16, tag="e")
                ets[idx] = (et, et.rearrange("p (c j) -> p c j", j=64))

            # ---- stage A: chunk-major wavefront over the first n_a passes.
            # Pass pa joins the wavefront at diagonal off[pa], matching its x
            # slab's DMA arrival so no x-gated cell blocks the in-order PE
            # queue; W2 builds are emitted two per diagonal, ahead of all
            # x2-dependent cells.
            off = TUNE["off"][:n_a]
            for idx in range(n_a):
                new_et(idx)
            for d in range(NGRP - 1 + (off[-1] if off else 0) + 1):
                if TUNE["builds_first"] and not (d == 0 and TUNE.get("d0_cells_first")):
                    for gb in (2 * d + 1, 2 * d + 2):
                        if gb < NGRP:
                            build_w2(gb)
                for pa in range(n_a - 1, -1, -1):
                    G = d - off[pa]
                    if not 0 <= G < NGRP:
                        continue
                    conv_group(G, proc_a[pa])
                    nsp = TUNE.get("ln_splits", 1)
                    per = NGRP // nsp
                    if (G + 1) % per == 0:
                        c0 = (G + 1 - per) * GRP
                        ln_part(proc_a[pa], c0, (G + 1) * GRP)
                    if G == NGRP - 1:
                        do_store(proc_a[pa])
                        if n_a + pa < len(plan):
                            issue_load(n_a + pa)
                if (not TUNE["builds_first"]) or (d == 0 and TUNE.get("d0_cells_first")):
                    for gb in (2 * d + 1, 2 * d + 2):
                        if gb < NGRP:
                            build_w2(gb)

            # ---- stage B: pass-major steady state (first loads were issued
            # at the end of stage A as x buffers freed)
            for idx in range(n_a, len(plan)):
                if idx + n_a < len(plan):
                    issue_load(idx + n_a)
                new_et(idx)
                for G in range(NGRP):
                    conv_group(G, idx)
                ln_store(idx)
    nc.finalize()
    return nc


def kernel(x: np.ndarray, kernels: np.ndarray) -> np.ndarray:
    assert x.shape == (B, T, C) and kernels.shape == (K, 1, C)
    ws1g, ws2g = _build_ws(np.asarray(kernels, dtype=np.float32))
    nc = build_nc(ws1g, ws2g)
    in_maps = []
    for i in range(N_CORES):
        xh = _prep_x(np.asarray(x[i * B_LOCAL:(i + 1) * B_LOCAL],
                                dtype=np.float32))
        in_maps.append(xh)
    res = run_bass_kernel_spmd(nc, in_maps, core_ids=list(range(N_CORES)))
    return np.concatenate([_post_y(r) for r in res.results], axis=0)


# revision 30
# speedup vs baseline: 1.0135x; 1.0135x over previous
"""Depthwise conv1d (128 channels, 128 taps, SAME) + softplus on 8 TRN2 cores.

Data-parallel over batch (16 -> 2 per core). Per channel the conv is two
banded-Toeplitz matmuls per 128-step output block (W1 on block j, W2 on block
j+1), accumulated in PSUM; softplus = Ln(Exp(y)+1) on the scalar engine (no
softplus table in this build).

All DMA is f16 with flat per-partition-contiguous access patterns on both
sides (the cost of a DMA is charged on its output AP, with a 2x penalty when
the innermost run is <512B, so layouts are chosen on the host):
  - x is converted to f16, padded by 64 on both ends of T, and pre-arranged on
    the host into per-pass slabs [128, (nb+1)*128] (partition = t mod 128).
  - y is written as f16 into a blocked DRAM layout [pass, p, (c j)] and
    rearranged/upcast to [B, T, C] f32 on the host.
  - W1 is f16, group-major, streamed in 16-channel chunks; W2 is built
    on-chip: one PE matmul per shift (stationary identity strip E, moving raw
    kernels) writes each slab to PSUM exactly, and DVE/GPSIMD copy it to SBUF
    f16 (bit-identical to a host-built table). This halves the startup table
    DMA, which would otherwise starve the activation engine.
The first three passes are processed channel-chunk-major along wavefront
diagonals so Exp work overlaps the W1/x streams; first and last passes are
small to shorten fill/drain. A single pre-placed act-table load (set 6:
natural_log_exp_and_others) covers both Exp and Ln (no table swaps).
"""
import numpy as np
import concourse.mybir as mybir
from concourse import bacc
from concourse.tile import TileContext
from concourse.bass_utils import run_bass_kernel_spmd

AF = mybir.ActivationFunctionType
N_CORES = 8
B, T, C, K = 16, 32768, 128, 128
B_LOCAL = B // N_CORES
NBLK = T // 128              # 256 output blocks per batch row
NGRP = 8                     # channel groups
GRP = C // NGRP              # 16 channels per group
N_STAGE_A = 3                # first passes processed chunk-major
ACT_SET_LN_EXP = 6           # natural_log_exp_and_others in act_info.json
TUNE = {"off": [0, 1, 3], "builds_first": True, "ln_splits": 1,
        "sizes0": [16, 32, 40, 64, 64, 40]}


def _pass_plan(b_local=B_LOCAL):
    """[(bb, start_block, nb), ...]; small first and last passes."""
    plan = []
    for bb in range(b_local):
        first, last = bb == 0, bb == b_local - 1
        if first and last:
            sizes = [16, 48, 64, 64, 48, 16]
        elif first:
            sizes = list(TUNE.get("sizes0", [16, 48, 64, 64, 64]))
        elif last:
            sizes = [64] * ((NBLK - 64) // 64) + [48, 16]
        else:
            sizes = [64] * (NBLK // 64)
        s = 0
        for nb in sizes:
            plan.append((bb, s, nb))
            s += nb
        assert s == NBLK
    return plan


def _build_ws(kernels_np: np.ndarray, dtype=np.float16):
    """Toeplitz tables, group-major: [NGRP, 128, 128*GRP] with
    ws[G][p][i*GRP + u] = T[p, i, G*GRP+u]."""
    w = kernels_np[:, 0, :].astype(np.float32)  # [k, c]
    p = np.arange(128)[:, None, None]
    i = np.arange(128)[None, :, None]
    c = np.arange(128)[None, None, :]
    k1 = p - i - 1
    k2 = p + 127 - i
    cb = np.broadcast_to(c, (128, 128, 128))
    T1 = np.where((k1 >= 0) & (k1 < K), w[np.clip(k1, 0, K - 1), cb], 0.0)
    T2 = np.where((k2 >= 0) & (k2 < K), w[np.clip(k2, 0, K - 1), cb], 0.0)

    def gm(t):  # [p, i, c] -> [G, p, i*GRP+u]
        t = t.reshape(128, 128, NGRP, GRP).transpose(2, 0, 1, 3)
        return np.ascontiguousarray(t.reshape(NGRP, 128, 128 * GRP)).astype(dtype)

    return gm(T1), gm(T2)


def _prep_x(x_np: np.ndarray):
    """[Bb, T, C] f32 -> {x<nb>: [n, 128, (nb+1)*128] f16 slabs}."""
    bb_n = x_np.shape[0]
    x16 = x_np.astype(np.float16)
    xpad = np.zeros((bb_n, T + 128, C), np.float16)
    xpad[:, 64:64 + T] = x16
    xj = xpad.reshape(bb_n, (T + 128) // 128, 128, C)    # [b, J, p, c]
    out = {}
    for bb, s, nb in _pass_plan(bb_n):
        slab = xj[bb, s:s + nb + 1]                      # [nb+1, p, c]
        slab = slab.transpose(1, 0, 2).reshape(128, -1)  # [p, (J c)]
        out.setdefault(f"x{nb}", []).append(slab)
    return {k: np.stack(v) for k, v in out.items()}


def _post_y(y_by_size, b_local=B_LOCAL) -> np.ndarray:
    """{y<nb>: [n, 128, C*nb] f16 blocked} -> [Bb, T, C] f32."""
    out = np.empty((b_local, T, C), np.float32)
    idx = {}
    for bb, s, nb in _pass_plan(b_local):
        i = idx.get(nb, 0)
        idx[nb] = i + 1
        blk = np.asarray(y_by_size[f"y{nb}"][i])          # [p, (c j)] f16
        blk = blk.reshape(128, C, nb).transpose(2, 0, 1)  # [j, p, c]
        out[bb, s * 128:(s + nb) * 128] = blk.reshape(nb * 128, C)
    return out


def build_nc(ws1g, ws2g, b_local=B_LOCAL, num_devices=N_CORES):
    f16, f32 = mybir.dt.float16, mybir.dt.float32
    CHUNK = 128 * GRP  # tile columns per ws group chunk
    plan = _pass_plan(b_local)
    n_a = min(N_STAGE_A, len(plan))
    counts = {}
    for _, _, nb in plan:
        counts[nb] = counts.get(nb, 0) + 1

    # raw kernels [k, c] f16, recovered exactly from the i=127 slab of W2
    w16_np = np.stack([ws2g[c // GRP][:, 127 * GRP + c % GRP]
                       for c in range(C)], axis=1).astype(np.float16)
    # identity strip: E[k, 128+k] = 1; W2 slab i is E[:, 255-i:383-i]^T @ w
    E_np = np.zeros((128, 384), np.float16)
    E_np[np.arange(128), np.arange(128) + 128] = 1.0

    nc = bacc.Bacc("TRN2", target_bir_lowering=False, debug=False,
                   num_devices=num_devices)
    xd = {nb: nc.dram_tensor(f"x{nb}", [n, 128, (nb + 1) * 128], f16,
                             kind="ExternalInput")
          for nb, n in counts.items()}
    yd = {nb: nc.dram_tensor(f"y{nb}", [n, 128, C * nb], f16,
                             kind="ExternalOutput")
          for nb, n in counts.items()}
    ws1_d = nc.inline_tensor(np.ascontiguousarray(ws1g), "ws1")
    ws2_d0 = nc.inline_tensor(np.ascontiguousarray(ws2g[0]), "ws2g0")
    e_d = nc.inline_tensor(E_np, "Estrip")
    w16_d = nc.inline_tensor(np.ascontiguousarray(w16_np), "w16")

    with TileContext(nc) as tc:
        with (
            tc.tile_pool(name="wpool", bufs=1) as wpool,
            tc.tile_pool(name="xpool", bufs=3) as xpool,
            tc.tile_pool(name="epool", bufs=3) as epool,
            tc.tile_pool(name="ypool", bufs=2) as ypool,
            tc.tile_pool(name="ppool", bufs=4, space="PSUM") as ppool,
        ):
            # Both Exp and Ln live in act set 6; one load => no table thrash.
            ld = mybir.InstLoadActFuncSet(
                name=nc.get_next_instruction_name(), ins=[], outs=[])
            ld.act_func_set_id = ACT_SET_LN_EXP
            ld.engine = mybir.EngineType.Activation
            nc.scalar.add_instruction(ld)

            ws1 = wpool.tile([128, NGRP * CHUNK], f16, tag="ws1")
            ws2 = wpool.tile([128, NGRP * CHUNK], f16, tag="ws2")
            et_strip = wpool.tile([128, 384], f16, tag="E")
            w16 = wpool.tile([128, 128], f16, tag="w16")

            nth = {}
            tiles = {}

            def issue_load(idx):
                bb, s, nb = plan[idx]
                i = nth.get(nb, 0)
                nth[nb] = i + 1
                xt = xpool.tile([128, 65 * 128], f16, tag="x")
                nc.sync.dma_start(out=xt[:, 0:(nb + 1) * 128], in_=xd[nb][i])
                tiles[idx] = (xt, i)

            def load_w1(G):
                nc.sync.dma_start(out=ws1[:, G * CHUNK:(G + 1) * CHUNK],
                                  in_=ws1_d.ap()[G])

            proc_a = TUNE.get("proc_a", list(range(n_a)))[:n_a]

            # JIT DMA order: E/w16 (for the W2 builds), then W1 chunks and x
            # slabs interleaved roughly by first use.
            nc.sync.dma_start(out=et_strip[:, :], in_=e_d.ap())
            nc.sync.dma_start(out=w16[:, :], in_=w16_d.ap())
            load_w1(0)
            nc.sync.dma_start(out=ws2[:, 0:CHUNK], in_=ws2_d0.ap())
            for tok in TUNE.get("prologue",
                                ["x:0", "x:1", "w1:1", "w1:2", "x:2", "w1:3",
                                 "w1:4", "w1:5", "w1:6", "w1:7"]):
                kind, _, num = tok.partition(":")
                if kind == "x":
                    issue_load(proc_a[int(num)])
                else:
                    load_w1(int(num))

            ws1v = ws1.rearrange("p (G i u) -> p G i u", G=NGRP, u=GRP)
            ws2v = ws2.rearrange("p (G i u) -> p G i u", G=NGRP, u=GRP)

            # PE p-state warmup: dummy matmuls on the identity strip while the
            # first DMAs are in flight, so the W2 builds and first conv run at
            # full clock instead of the cold 0.65 GHz p-state.
            for _ in range(TUNE.get("warmup_mms", 0)):
                psd = ppool.tile([128, GRP * 64], f32, tag="ps")
                nc.tensor.matmul(psd[:, 0:384], et_strip[:, 0:128],
                                 et_strip[:, 0:384], start=True, stop=True)

            def build_w2(G):
                # 128 exact-shift matmuls fill chunk G of W2; two PSUM halves
                # copied to SBUF by DVE and GPSIMD in parallel.
                for half in range(2):
                    ps2 = ppool.tile([128, GRP * 64], f32, tag="ps")
                    for ii in range(64):
                        i = half * 64 + ii
                        nc.tensor.matmul(ps2[:, ii * GRP:(ii + 1) * GRP],
                                         et_strip[:, 255 - i:383 - i],
                                         w16[:, G * GRP:(G + 1) * GRP],
                                         start=True, stop=True)
                    lo = G * CHUNK + half * 64 * GRP
                    nc.vector.tensor_copy(out=ws2[:, lo:lo + 64 * GRP],
                                          in_=ps2[:, :])

            def conv_group(G, idx):
                bb, s, nb = plan[idx]
                xt, _ = tiles[idx]
                x3 = xt.rearrange("p (j c) -> p j c", c=128)
                et, e3 = ets[idx]
                ps = ppool.tile([128, GRP * 64], f32, tag="ps")
                for u in range(GRP):
                    ch = G * GRP + u
                    nc.tensor.matmul(ps[:, u * 64:u * 64 + nb],
                                     ws1v[:, G, :, u], x3[:, 0:nb, ch],
                                     start=True, stop=False)
                    nc.tensor.matmul(ps[:, u * 64:u * 64 + nb],
                                     ws2v[:, G, :, u], x3[:, 1:nb + 1, ch],
                                     start=False, stop=True)
                nc.scalar.activation(
                    e3[:, G * GRP:(G + 1) * GRP, 0:nb],
                    ps.rearrange("p (u j) -> p u j", j=64)[:, :, 0:nb],
                    AF.Exp)

            yts = {}

            def ln_part(idx, c0, c1):
                bb, s, nb = plan[idx]
                et, e3 = ets[idx]
                if idx not in yts:
                    if nb == 16:
                        yt_new = ypool.tile([128, C * 16], f16, tag="ysm")
                    else:
                        yt_new = ypool.tile([128, C * 64], f16, tag="y")
                    yts[idx] = yt_new
                yt = yts[idx]
                if nb == 64:
                    nc.scalar.activation(yt[:, c0 * 64:c1 * 64],
                                         et[:, c0 * 64:c1 * 64],
                                         AF.Ln, bias=1.0)
                else:
                    y3 = yt[:, c0 * nb:c1 * nb].rearrange(
                        "p (c j) -> p c j", j=nb)
                    nc.scalar.activation(y3[:, :, :], e3[:, c0:c1, 0:nb],
                                         AF.Ln, bias=1.0)

            def do_store(idx):
                bb, s, nb = plan[idx]
                _, yi = tiles.pop(idx)
                ets.pop(idx)
                # last store on the HWDGE path: shorter tail before final drain
                eng = nc.sync if idx == len(plan) - 1 else nc.gpsimd
                yt = yts.pop(idx)
                eng.dma_start(out=yd[nb][yi], in_=yt[:, 0:C * nb])

            def ln_store(idx):
                bb, s, nb = plan[idx]
                if idx == len(plan) - 1:
                    # overlap the final store with the second Ln half so the
                    # end-of-kernel drain waits on a half-size transfer
                    ln_part(idx, 0, C // 2)
                    _, yi = tiles[idx]
                    yt = yts[idx]
                    nc.sync.dma_start(out=yd[nb][yi][:, 0:(C // 2) * nb],
                                      in_=yt[:, 0:(C // 2) * nb])
                    ln_part(idx, C // 2, C)
                    tiles.pop(idx)
                    ets.pop(idx)
                    yts.pop(idx)
                    nc.sync.dma_start(
                        out=yd[nb][yi][:, (C // 2) * nb:C * nb],
                        in_=yt[:, (C // 2) * nb:C * nb])
                else:
                    ln_part(idx, 0, C)
                    do_store(idx)

            ets = {}

            def new_et(idx):
                et = epool.tile([128, C * 64], f16, tag="e")
                ets[idx] = (et, et.rearrange("p (c j) -> p c j", j=64))

            # ---- stage A: chunk-major wavefront over the first n_a passes.
            # Pass pa joins the wavefront at diagonal off[pa], matching its x
            # slab's DMA arrival so no x-gated cell blocks the in-order PE
            # queue; W2 builds are emitted two per diagonal, ahead of all
            # x2-dependent cells.
            off = TUNE["off"][:n_a]
            for idx in range(n_a):
                new_et(idx)
            for d in range(NGRP - 1 + (off[-1] if off else 0) + 1):
                if TUNE["builds_first"] and not (d == 0 and TUNE.get("d0_cells_first")):
                    for gb in (2 * d + 1, 2 * d + 2):
                        if gb < NGRP:
                            build_w2(gb)
                for pa in range(n_a - 1, -1, -1):
                    G = d - off[pa]
                    if not 0 <= G < NGRP:
                        continue
                    conv_group(G, proc_a[pa])
                    nsp = TUNE.get("ln_splits", 1)
                    per = NGRP // nsp
                    if (G + 1) % per == 0:
                        c0 = (G + 1 - per) * GRP
                        ln_part(proc_a[pa], c0, (G + 1) * GRP)
                    if G == NGRP - 1:
                        do_store(proc_a[pa])
                        if n_a + pa < len(plan):
                            issue_load(n_a + pa)
                if (not TUNE["builds_first"]) or (d == 0 and TUNE.get("d0_cells_first")):
                    for gb in (2 * d + 1, 2 * d + 2):
                        if gb < NGRP:
                            build_w2(gb)

            # ---- stage B: pass-major steady state (first loads were issued
            # at the end of stage A as x buffers freed)
            for idx in range(n_a, len(plan)):
                if idx + n_a < len(plan):
                    issue_load(idx + n_a)
                new_et(idx)
                for G in range(NGRP):
                    conv_group(G, idx)
                ln_store(idx)
    nc.finalize()
    return nc


def kernel(x: np.ndarray, kernels: np.ndarray) -> np.ndarray:
    assert x.shape == (B, T, C) and kernels.shape == (K, 1, C)
    ws1g, ws2g = _build_ws(np.asarray(kernels, dtype=np.float32))
    nc = build_nc(ws1g, ws2g)
    in_maps = []
    for i in range(N_CORES):
        xh = _prep_x(np.asarray(x[i * B_LOCAL:(i + 1) * B_LOCAL],
                                dtype=np.float32))
        in_maps.append(xh)
    res = run_bass_kernel_spmd(nc, in_maps, core_ids=list(range(N_CORES)))
    return np.concatenate([_post_y(r) for r in res.results], axis=0)


# revision 37
# speedup vs baseline: 1.0179x; 1.0044x over previous
"""Depthwise conv1d (128 channels, 128 taps, SAME) + softplus on 8 TRN2 cores.

Data-parallel over batch (16 -> 2 per core). Per channel the conv is two
banded-Toeplitz matmuls per 128-step output block (W1 on block j, W2 on block
j+1), accumulated in PSUM; softplus = Ln(Exp(y)+1) on the scalar engine (no
softplus table in this build).

All DMA is f16 with flat per-partition-contiguous access patterns on both
sides (the cost of a DMA is charged on its output AP, with a 2x penalty when
the innermost run is <512B, so layouts are chosen on the host):
  - x is converted to f16, padded by 64 on both ends of T, and pre-arranged on
    the host into per-pass slabs [128, (nb+1)*128] (partition = t mod 128).
  - y is written as f16 into a blocked DRAM layout [pass, p, (c j)] and
    rearranged/upcast to [B, T, C] f32 on the host.
  - W1 is f16, group-major, streamed in 16-channel chunks; W2 is built
    on-chip: one PE matmul per shift (stationary identity strip E, moving raw
    kernels) writes each slab to PSUM exactly, and DVE/GPSIMD copy it to SBUF
    f16 (bit-identical to a host-built table). This halves the startup table
    DMA, which would otherwise starve the activation engine.
The first three passes are processed channel-chunk-major along wavefront
diagonals so Exp work overlaps the W1/x streams; first and last passes are
small to shorten fill/drain. A single pre-placed act-table load (set 6:
natural_log_exp_and_others) covers both Exp and Ln (no table swaps).
"""
import numpy as np
import concourse.mybir as mybir
from concourse import bacc
from concourse.tile import TileContext
from concourse.bass_utils import run_bass_kernel_spmd

AF = mybir.ActivationFunctionType
N_CORES = 8
B, T, C, K = 16, 32768, 128, 128
B_LOCAL = B // N_CORES
NBLK = T // 128              # 256 output blocks per batch row
NGRP = 8                     # channel groups
GRP = C // NGRP              # 16 channels per group
N_STAGE_A = 3                # first passes processed chunk-major
ACT_SET_LN_EXP = 6           # natural_log_exp_and_others in act_info.json
TUNE = {"off": [0, 1, 3], "builds_first": True, "ln_splits": 1,
        "sizes0": [16, 32, 40, 64, 64, 40],
        "sizes1": [64, 64, 64, 46, 18]}


def _pass_plan(b_local=B_LOCAL):
    """[(bb, start_block, nb), ...]; small first and last passes."""
    plan = []
    for bb in range(b_local):
        first, last = bb == 0, bb == b_local - 1
        if first and last:
            sizes = [16, 48, 64, 64, 48, 16]
        elif first:
            sizes = list(TUNE.get("sizes0", [16, 48, 64, 64, 64]))
        elif last:
            sizes = list(TUNE.get("sizes1", [64, 64, 64, 48, 16]))
        else:
            sizes = [64] * (NBLK // 64)
        s = 0
        for nb in sizes:
            plan.append((bb, s, nb))
            s += nb
        assert s == NBLK
    return plan


def _build_ws(kernels_np: np.ndarray, dtype=np.float16):
    """Toeplitz tables, group-major: [NGRP, 128, 128*GRP] with
    ws[G][p][i*GRP + u] = T[p, i, G*GRP+u]."""
    w = kernels_np[:, 0, :].astype(np.float32)  # [k, c]
    p = np.arange(128)[:, None, None]
    i = np.arange(128)[None, :, None]
    c = np.arange(128)[None, None, :]
    k1 = p - i - 1
    k2 = p + 127 - i
    cb = np.broadcast_to(c, (128, 128, 128))
    T1 = np.where((k1 >= 0) & (k1 < K), w[np.clip(k1, 0, K - 1), cb], 0.0)
    T2 = np.where((k2 >= 0) & (k2 < K), w[np.clip(k2, 0, K - 1), cb], 0.0)

    def gm(t):  # [p, i, c] -> [G, p, i*GRP+u]
        t = t.reshape(128, 128, NGRP, GRP).transpose(2, 0, 1, 3)
        return np.ascontiguousarray(t.reshape(NGRP, 128, 128 * GRP)).astype(dtype)

    return gm(T1), gm(T2)


def _prep_x(x_np: np.ndarray):
    """[Bb, T, C] f32 -> {x<nb>: [n, 128, (nb+1)*128] f16 slabs}."""
    bb_n = x_np.shape[0]
    x16 = x_np.astype(np.float16)
    xpad = np.zeros((bb_n, T + 128, C), np.float16)
    xpad[:, 64:64 + T] = x16
    xj = xpad.reshape(bb_n, (T + 128) // 128, 128, C)    # [b, J, p, c]
    out = {}
    for bb, s, nb in _pass_plan(bb_n):
        slab = xj[bb, s:s + nb + 1]                      # [nb+1, p, c]
        slab = slab.transpose(1, 0, 2).reshape(128, -1)  # [p, (J c)]
        out.setdefault(f"x{nb}", []).append(slab)
    return {k: np.stack(v) for k, v in out.items()}


def _post_y(y_by_size, b_local=B_LOCAL) -> np.ndarray:
    """{y<nb>: [n, 128, C*nb] f16 blocked} -> [Bb, T, C] f32."""
    out = np.empty((b_local, T, C), np.float32)
    idx = {}
    for bb, s, nb in _pass_plan(b_local):
        i = idx.get(nb, 0)
        idx[nb] = i + 1
        blk = np.asarray(y_by_size[f"y{nb}"][i])          # [p, (c j)] f16
        blk = blk.reshape(128, C, nb).transpose(2, 0, 1)  # [j, p, c]
        out[bb, s * 128:(s + nb) * 128] = blk.reshape(nb * 128, C)
    return out


def build_nc(ws1g, ws2g, b_local=B_LOCAL, num_devices=N_CORES):
    f16, f32 = mybir.dt.float16, mybir.dt.float32
    CHUNK = 128 * GRP  # tile columns per ws group chunk
    plan = _pass_plan(b_local)
    n_a = min(N_STAGE_A, len(plan))
    counts = {}
    for _, _, nb in plan:
        counts[nb] = counts.get(nb, 0) + 1

    # raw kernels [k, c] f16, recovered exactly from the i=127 slab of W2
    w16_np = np.stack([ws2g[c // GRP][:, 127 * GRP + c % GRP]
                       for c in range(C)], axis=1).astype(np.float16)
    # identity strip: E[k, 128+k] = 1; W2 slab i is E[:, 255-i:383-i]^T @ w
    E_np = np.zeros((128, 384), np.float16)
    E_np[np.arange(128), np.arange(128) + 128] = 1.0

    nc = bacc.Bacc("TRN2", target_bir_lowering=False, debug=False,
                   num_devices=num_devices)
    xd = {nb: nc.dram_tensor(f"x{nb}", [n, 128, (nb + 1) * 128], f16,
                             kind="ExternalInput")
          for nb, n in counts.items()}
    yd = {nb: nc.dram_tensor(f"y{nb}", [n, 128, C * nb], f16,
                             kind="ExternalOutput")
          for nb, n in counts.items()}
    ws1_d = nc.inline_tensor(np.ascontiguousarray(ws1g), "ws1")
    ws2_d0 = nc.inline_tensor(np.ascontiguousarray(ws2g[0]), "ws2g0")
    e_d = nc.inline_tensor(E_np, "Estrip")
    w16_d = nc.inline_tensor(np.ascontiguousarray(w16_np), "w16")

    with TileContext(nc) as tc:
        with (
            tc.tile_pool(name="wpool", bufs=1) as wpool,
            tc.tile_pool(name="xpool", bufs=3) as xpool,
            tc.tile_pool(name="epool", bufs=3) as epool,
            tc.tile_pool(name="ypool", bufs=2) as ypool,
            tc.tile_pool(name="ppool", bufs=4, space="PSUM") as ppool,
        ):
            # Both Exp and Ln live in act set 6; one load => no table thrash.
            ld = mybir.InstLoadActFuncSet(
                name=nc.get_next_instruction_name(), ins=[], outs=[])
            ld.act_func_set_id = ACT_SET_LN_EXP
            ld.engine = mybir.EngineType.Activation
            nc.scalar.add_instruction(ld)

            ws1 = wpool.tile([128, NGRP * CHUNK], f16, tag="ws1")
            ws2 = wpool.tile([128, NGRP * CHUNK], f16, tag="ws2")
            et_strip = wpool.tile([128, 384], f16, tag="E")
            w16 = wpool.tile([128, 128], f16, tag="w16")

            nth = {}
            tiles = {}

            def issue_load(idx):
                bb, s, nb = plan[idx]
                i = nth.get(nb, 0)
                nth[nb] = i + 1
                xt = xpool.tile([128, 65 * 128], f16, tag="x")
                nc.sync.dma_start(out=xt[:, 0:(nb + 1) * 128], in_=xd[nb][i])
                tiles[idx] = (xt, i)

            def load_w1(G):
                nc.sync.dma_start(out=ws1[:, G * CHUNK:(G + 1) * CHUNK],
                                  in_=ws1_d.ap()[G])

            proc_a = TUNE.get("proc_a", list(range(n_a)))[:n_a]

            # JIT DMA order: E/w16 (for the W2 builds), then W1 chunks and x
            # slabs interleaved roughly by first use.
            for tok in TUNE.get("prologue",
                                ["E", "w16", "w1:0", "w2g0",
                                 "x:0", "w1:1", "x:1", "w1:2", "x:2", "w1:3",
                                 "w1:4", "w1:5", "w1:6", "w1:7"]):
                kind, _, num = tok.partition(":")
                if kind == "x":
                    issue_load(proc_a[int(num)])
                elif kind == "E":
                    nc.sync.dma_start(out=et_strip[:, :], in_=e_d.ap())
                elif kind == "w16":
                    nc.sync.dma_start(out=w16[:, :], in_=w16_d.ap())
                elif kind == "w2g0":
                    nc.sync.dma_start(out=ws2[:, 0:CHUNK], in_=ws2_d0.ap())
                else:
                    load_w1(int(num))

            ws1v = ws1.rearrange("p (G i u) -> p G i u", G=NGRP, u=GRP)
            ws2v = ws2.rearrange("p (G i u) -> p G i u", G=NGRP, u=GRP)

            # PE p-state warmup: dummy matmuls on the identity strip while the
            # first DMAs are in flight, so the W2 builds and first conv run at
            # full clock instead of the cold 0.65 GHz p-state.
            for _ in range(TUNE.get("warmup_mms", 0)):
                psd = ppool.tile([128, GRP * 64], f32, tag="ps")
                nc.tensor.matmul(psd[:, 0:384], et_strip[:, 0:128],
                                 et_strip[:, 0:384], start=True, stop=True)

            def build_w2(G):
                # 128 exact-shift matmuls fill chunk G of W2; two PSUM halves
                # copied to SBUF by DVE and GPSIMD in parallel.
                for half in range(2):
                    ps2 = ppool.tile([128, GRP * 64], f32, tag="ps")
                    for ii in range(64):
                        i = half * 64 + ii
                        nc.tensor.matmul(ps2[:, ii * GRP:(ii + 1) * GRP],
                                         et_strip[:, 255 - i:383 - i],
                                         w16[:, G * GRP:(G + 1) * GRP],
                                         start=True, stop=True)
                    lo = G * CHUNK + half * 64 * GRP
                    nc.vector.tensor_copy(out=ws2[:, lo:lo + 64 * GRP],
                                          in_=ps2[:, :])

            def conv_group(G, idx):
                bb, s, nb = plan[idx]
                xt, _ = tiles[idx]
                x3 = xt.rearrange("p (j c) -> p j c", c=128)
                et, e3 = ets[idx]
                ps = ppool.tile([128, GRP * 64], f32, tag="ps")
                for u in range(GRP):
                    ch = G * GRP + u
                    nc.tensor.matmul(ps[:, u * 64:u * 64 + nb],
                                     ws1v[:, G, :, u], x3[:, 0:nb, ch],
                                     start=True, stop=False)
                    nc.tensor.matmul(ps[:, u * 64:u * 64 + nb],
                                     ws2v[:, G, :, u], x3[:, 1:nb + 1, ch],
                                     start=False, stop=True)
                nc.scalar.activation(
                    e3[:, G * GRP:(G + 1) * GRP, 0:nb],
                    ps.rearrange("p (u j) -> p u j", j=64)[:, :, 0:nb],
                    AF.Exp)

            yts = {}

            def ln_part(idx, c0, c1):
                bb, s, nb = plan[idx]
                et, e3 = ets[idx]
                if idx not in yts:
                    if nb == 16:
                        yt_new = ypool.tile([128, C * 16], f16, tag="ysm")
                    else:
                        yt_new = ypool.tile([128, C * 64], f16, tag="y")
                    yts[idx] = yt_new
                yt = yts[idx]
                if nb == 64:
                    nc.scalar.activation(yt[:, c0 * 64:c1 * 64],
                                         et[:, c0 * 64:c1 * 64],
                                         AF.Ln, bias=1.0)
                else:
                    y3 = yt[:, c0 * nb:c1 * nb].rearrange(
                        "p (c j) -> p c j", j=nb)
                    nc.scalar.activation(y3[:, :, :], e3[:, c0:c1, 0:nb],
                                         AF.Ln, bias=1.0)

            def do_store(idx):
                bb, s, nb = plan[idx]
                _, yi = tiles.pop(idx)
                ets.pop(idx)
                # last store on the HWDGE path: shorter tail before final drain
                eng = nc.sync if idx == len(plan) - 1 else nc.gpsimd
                yt = yts.pop(idx)
                eng.dma_start(out=yd[nb][yi], in_=yt[:, 0:C * nb])

            def ln_store(idx):
                bb, s, nb = plan[idx]
                if idx == len(plan) - 1:
                    # overlap the final store with the second Ln half so the
                    # end-of-kernel drain waits on a half-size transfer
                    ln_part(idx, 0, C // 2)
                    _, yi = tiles[idx]
                    yt = yts[idx]
                    nc.sync.dma_start(out=yd[nb][yi][:, 0:(C // 2) * nb],
                                      in_=yt[:, 0:(C // 2) * nb])
                    ln_part(idx, C // 2, C)
                    tiles.pop(idx)
                    ets.pop(idx)
                    yts.pop(idx)
                    nc.sync.dma_start(
                        out=yd[nb][yi][:, (C // 2) * nb:C * nb],
                        in_=yt[:, (C // 2) * nb:C * nb])
                else:
                    ln_part(idx, 0, C)
                    do_store(idx)

            ets = {}

            def new_et(idx):
                et = epool.tile([128, C * 64], f16, tag="e")
                ets[idx] = (et, et.rearrange("p (c j) -> p c j", j=64))

            # ---- stage A: chunk-major wavefront over the first n_a passes.
            # Pass pa joins the wavefront at diagonal off[pa], matching its x
            # slab's DMA arrival so no x-gated cell blocks the in-order PE
            # queue; W2 builds are emitted two per diagonal, ahead of all
            # x2-dependent cells.
            off = TUNE["off"][:n_a]
            for idx in range(n_a):
                new_et(idx)
            for d in range(NGRP - 1 + (off[-1] if off else 0) + 1):
                if TUNE["builds_first"] and not (d == 0 and TUNE.get("d0_cells_first")):
                    for gb in (2 * d + 1, 2 * d + 2):
                        if gb < NGRP:
                            build_w2(gb)
                for pa in range(n_a - 1, -1, -1):
                    G = d - off[pa]
                    if not 0 <= G < NGRP:
                        continue
                    conv_group(G, proc_a[pa])
                    nsp = TUNE.get("ln_splits", 1)
                    per = NGRP // nsp
                    if (G + 1) % per == 0:
                        c0 = (G + 1 - per) * GRP
                        ln_part(proc_a[pa], c0, (G + 1) * GRP)
                    if G == NGRP - 1:
                        do_store(proc_a[pa])
                        if n_a + pa < len(plan):
                            issue_load(n_a + pa)
                if (not TUNE["builds_first"]) or (d == 0 and TUNE.get("d0_cells_first")):
                    for gb in (2 * d + 1, 2 * d + 2):
                        if gb < NGRP:
                            build_w2(gb)

            # ---- stage B: pass-major steady state (first loads were issued
            # at the end of stage A as x buffers freed)
            for idx in range(n_a, len(plan)):
                if idx + n_a < len(plan):
                    issue_load(idx + n_a)
                new_et(idx)
                for G in range(NGRP):
                    conv_group(G, idx)
                ln_store(idx)
    nc.finalize()
    return nc


def kernel(x: np.ndarray, kernels: np.ndarray) -> np.ndarray:
    assert x.shape == (B, T, C) and kernels.shape == (K, 1, C)
    ws1g, ws2g = _build_ws(np.asarray(kernels, dtype=np.float32))
    nc = build_nc(ws1g, ws2g)
    in_maps = []
    for i in range(N_CORES):
        xh = _prep_x(np.asarray(x[i * B_LOCAL:(i + 1) * B_LOCAL],
                                dtype=np.float32))
        in_maps.append(xh)
    res = run_bass_kernel_spmd(nc, in_maps, core_ids=list(range(N_CORES)))
    return np.concatenate([_post_y(r) for r in res.results], axis=0)


# revision 41
# speedup vs baseline: 1.0226x; 1.0046x over previous
"""Depthwise conv1d (128 channels, 128 taps, SAME) + softplus on 8 TRN2 cores.

Data-parallel over batch (16 -> 2 per core). Per channel the conv is two
banded-Toeplitz matmuls per 128-step output block (W1 on block j, W2 on block
j+1), accumulated in PSUM; softplus = Ln(Exp(y)+1) on the scalar engine (no
softplus table in this build).

All DMA is f16 with flat per-partition-contiguous access patterns on both
sides (the cost of a DMA is charged on its output AP, with a 2x penalty when
the innermost run is <512B, so layouts are chosen on the host):
  - x is converted to f16, padded by 64 on both ends of T, and pre-arranged on
    the host into per-pass slabs [128, (nb+1)*128] (partition = t mod 128).
  - y is written as f16 into a blocked DRAM layout [pass, p, (c j)] and
    rearranged/upcast to [B, T, C] f32 on the host.
  - W1 is f16, group-major, streamed in 16-channel chunks; W2 is built
    on-chip: one PE matmul per shift (stationary identity strip E, moving raw
    kernels) writes each slab to PSUM exactly, and DVE/GPSIMD copy it to SBUF
    f16 (bit-identical to a host-built table). This halves the startup table
    DMA, which would otherwise starve the activation engine.
The first three passes are processed channel-chunk-major along wavefront
diagonals so Exp work overlaps the W1/x streams; first and last passes are
small to shorten fill/drain. A single pre-placed act-table load (set 6:
natural_log_exp_and_others) covers both Exp and Ln (no table swaps).
"""
import numpy as np
import concourse.mybir as mybir
from concourse import bacc
from concourse.tile import TileContext
from concourse.bass_utils import run_bass_kernel_spmd

AF = mybir.ActivationFunctionType
N_CORES = 8
B, T, C, K = 16, 32768, 128, 128
B_LOCAL = B // N_CORES
NBLK = T // 128              # 256 output blocks per batch row
NGRP = 8                     # channel groups
GRP = C // NGRP              # 16 channels per group
N_STAGE_A = 3                # first passes processed chunk-major
ACT_SET_LN_EXP = 6           # natural_log_exp_and_others in act_info.json
TUNE = {"off": [0, 1, 3], "builds_first": True, "ln_splits": 1,
        "sizes0": [16, 32, 36, 64, 64, 44],
        "sizes1": [64, 64, 64, 46, 18]}


def _pass_plan(b_local=B_LOCAL):
    """[(bb, start_block, nb), ...]; small first and last passes."""
    plan = []
    for bb in range(b_local):
        first, last = bb == 0, bb == b_local - 1
        if first and last:
            sizes = [16, 48, 64, 64, 48, 16]
        elif first:
            sizes = list(TUNE.get("sizes0", [16, 48, 64, 64, 64]))
        elif last:
            sizes = list(TUNE.get("sizes1", [64, 64, 64, 48, 16]))
        else:
            sizes = [64] * (NBLK // 64)
        s = 0
        for nb in sizes:
            plan.append((bb, s, nb))
            s += nb
        assert s == NBLK
    return plan


def _build_ws(kernels_np: np.ndarray, dtype=np.float16):
    """Toeplitz tables, group-major: [NGRP, 128, 128*GRP] with
    ws[G][p][i*GRP + u] = T[p, i, G*GRP+u]."""
    w = kernels_np[:, 0, :].astype(np.float32)  # [k, c]
    p = np.arange(128)[:, None, None]
    i = np.arange(128)[None, :, None]
    c = np.arange(128)[None, None, :]
    k1 = p - i - 1
    k2 = p + 127 - i
    cb = np.broadcast_to(c, (128, 128, 128))
    T1 = np.where((k1 >= 0) & (k1 < K), w[np.clip(k1, 0, K - 1), cb], 0.0)
    T2 = np.where((k2 >= 0) & (k2 < K), w[np.clip(k2, 0, K - 1), cb], 0.0)

    def gm(t):  # [p, i, c] -> [G, p, i*GRP+u]
        t = t.reshape(128, 128, NGRP, GRP).transpose(2, 0, 1, 3)
        return np.ascontiguousarray(t.reshape(NGRP, 128, 128 * GRP)).astype(dtype)

    return gm(T1), gm(T2)


def _prep_x(x_np: np.ndarray):
    """[Bb, T, C] f32 -> {x<nb>: [n, 128, (nb+1)*128] f16 slabs}."""
    bb_n = x_np.shape[0]
    x16 = x_np.astype(np.float16)
    xpad = np.zeros((bb_n, T + 128, C), np.float16)
    xpad[:, 64:64 + T] = x16
    xj = xpad.reshape(bb_n, (T + 128) // 128, 128, C)    # [b, J, p, c]
    out = {}
    for bb, s, nb in _pass_plan(bb_n):
        slab = xj[bb, s:s + nb + 1]                      # [nb+1, p, c]
        slab = slab.transpose(1, 0, 2).reshape(128, -1)  # [p, (J c)]
        out.setdefault(f"x{nb}", []).append(slab)
    return {k: np.stack(v) for k, v in out.items()}


def _post_y(y_by_size, b_local=B_LOCAL) -> np.ndarray:
    """{y<nb>: [n, 128, C*nb] f16 blocked} -> [Bb, T, C] f32."""
    out = np.empty((b_local, T, C), np.float32)
    idx = {}
    for bb, s, nb in _pass_plan(b_local):
        i = idx.get(nb, 0)
        idx[nb] = i + 1
        blk = np.asarray(y_by_size[f"y{nb}"][i])          # [p, (c j)] f16
        blk = blk.reshape(128, C, nb).transpose(2, 0, 1)  # [j, p, c]
        out[bb, s * 128:(s + nb) * 128] = blk.reshape(nb * 128, C)
    return out


def build_nc(ws1g, ws2g, b_local=B_LOCAL, num_devices=N_CORES):
    f16, f32 = mybir.dt.float16, mybir.dt.float32
    CHUNK = 128 * GRP  # tile columns per ws group chunk
    plan = _pass_plan(b_local)
    n_a = min(N_STAGE_A, len(plan))
    counts = {}
    for _, _, nb in plan:
        counts[nb] = counts.get(nb, 0) + 1

    # raw kernels [k, c] f16, recovered exactly from the i=127 slab of W2
    w16_np = np.stack([ws2g[c // GRP][:, 127 * GRP + c % GRP]
                       for c in range(C)], axis=1).astype(np.float16)
    # identity strip: E[k, 128+k] = 1; W2 slab i is E[:, 255-i:383-i]^T @ w.
    # Packed with the raw kernels into one tensor so startup is a single DMA.
    EW_np = np.zeros((128, 512), np.float16)
    EW_np[np.arange(128), np.arange(128) + 128] = 1.0
    EW_np[:, 384:512] = w16_np

    nc = bacc.Bacc("TRN2", target_bir_lowering=False, debug=False,
                   num_devices=num_devices)
    xd = {nb: nc.dram_tensor(f"x{nb}", [n, 128, (nb + 1) * 128], f16,
                             kind="ExternalInput")
          for nb, n in counts.items()}
    yd = {nb: nc.dram_tensor(f"y{nb}", [n, 128, C * nb], f16,
                             kind="ExternalOutput")
          for nb, n in counts.items()}
    ws1_d = nc.inline_tensor(np.ascontiguousarray(ws1g), "ws1")
    ws2_d0 = nc.inline_tensor(np.ascontiguousarray(ws2g[0]), "ws2g0")
    ew_d = nc.inline_tensor(EW_np, "EW")

    with TileContext(nc) as tc:
        with (
            tc.tile_pool(name="wpool", bufs=1) as wpool,
            tc.tile_pool(name="xpool", bufs=3) as xpool,
            tc.tile_pool(name="epool", bufs=3) as epool,
            tc.tile_pool(name="ypool", bufs=2) as ypool,
            tc.tile_pool(name="ppool", bufs=4, space="PSUM") as ppool,
        ):
            # Both Exp and Ln live in act set 6; one load => no table thrash.
            ld = mybir.InstLoadActFuncSet(
                name=nc.get_next_instruction_name(), ins=[], outs=[])
            ld.act_func_set_id = ACT_SET_LN_EXP
            ld.engine = mybir.EngineType.Activation
            nc.scalar.add_instruction(ld)

            ws1 = wpool.tile([128, NGRP * CHUNK], f16, tag="ws1")
            ws2 = wpool.tile([128, NGRP * CHUNK], f16, tag="ws2")
            ew = wpool.tile([128, 512], f16, tag="EW")
            et_strip = ew[:, 0:384]
            w16 = ew[:, 384:512]

            nth = {}
            tiles = {}

            def issue_load(idx):
                bb, s, nb = plan[idx]
                i = nth.get(nb, 0)
                nth[nb] = i + 1
                xt = xpool.tile([128, 65 * 128], f16, tag="x")
                nc.sync.dma_start(out=xt[:, 0:(nb + 1) * 128], in_=xd[nb][i])
                tiles[idx] = (xt, i)

            def load_w1(G):
                nc.sync.dma_start(out=ws1[:, G * CHUNK:(G + 1) * CHUNK],
                                  in_=ws1_d.ap()[G])

            proc_a = TUNE.get("proc_a", list(range(n_a)))[:n_a]

            # JIT DMA order: E/w16 (for the W2 builds), then W1 chunks and x
            # slabs interleaved roughly by first use.
            for tok in TUNE.get("prologue",
                                ["E", "w16", "w1:0", "w2g0",
                                 "x:0", "w1:1", "x:1", "w1:2", "x:2", "w1:3",
                                 "w1:4", "w1:5", "w1:6", "w1:7"]):
                kind, _, num = tok.partition(":")
                if kind == "x":
                    issue_load(proc_a[int(num)])
                elif kind == "E":
                    nc.sync.dma_start(out=ew[:, :], in_=ew_d.ap())
                elif kind == "w16":
                    pass  # merged into the EW load
                elif kind == "w2g0":
                    nc.sync.dma_start(out=ws2[:, 0:CHUNK], in_=ws2_d0.ap())
                elif int(num) in TUNE.get("w1_build", ()):
                    pass  # this W1 chunk is PE-built, not DMA-loaded
                else:
                    load_w1(int(num))

            ws1v = ws1.rearrange("p (G i u) -> p G i u", G=NGRP, u=GRP)
            ws2v = ws2.rearrange("p (G i u) -> p G i u", G=NGRP, u=GRP)

            # PE p-state warmup: dummy matmuls on the identity strip while the
            # first DMAs are in flight, so the W2 builds and first conv run at
            # full clock instead of the cold 0.65 GHz p-state.
            for _ in range(TUNE.get("warmup_mms", 0)):
                psd = ppool.tile([128, GRP * 64], f32, tag="ps")
                nc.tensor.matmul(psd[:, 0:384], et_strip[:, 0:128],
                                 et_strip[:, 0:384], start=True, stop=True)

            def build_tbl(G, dst, a_of_i):
                # 128 exact-shift matmuls fill chunk G; PSUM halves copied to
                # SBUF by DVE (bit-identical to a host-built table).
                for half in range(2):
                    ps2 = ppool.tile([128, GRP * 64], f32, tag="ps")
                    for ii in range(64):
                        i = half * 64 + ii
                        a = a_of_i(i)
                        nc.tensor.matmul(ps2[:, ii * GRP:(ii + 1) * GRP],
                                         et_strip[:, a:a + 128],
                                         w16[:, G * GRP:(G + 1) * GRP],
                                         start=True, stop=True)
                    lo = G * CHUNK + half * 64 * GRP
                    nc.vector.tensor_copy(out=dst[:, lo:lo + 64 * GRP],
                                          in_=ps2[:, :])

            def build_w2(G):
                build_tbl(G, ws2, lambda i: 255 - i)

            def build_w1(G):
                build_tbl(G, ws1, lambda i: 127 - i)

            def conv_group(G, idx):
                bb, s, nb = plan[idx]
                xt, _ = tiles[idx]
                x3 = xt.rearrange("p (j c) -> p j c", c=128)
                et, e3 = ets[idx]
                ps = ppool.tile([128, GRP * 64], f32, tag="ps")
                for u in range(GRP):
                    ch = G * GRP + u
                    nc.tensor.matmul(ps[:, u * 64:u * 64 + nb],
                                     ws1v[:, G, :, u], x3[:, 0:nb, ch],
                                     start=True, stop=False)
                    nc.tensor.matmul(ps[:, u * 64:u * 64 + nb],
                                     ws2v[:, G, :, u], x3[:, 1:nb + 1, ch],
                                     start=False, stop=True)
                nc.scalar.activation(
                    e3[:, G * GRP:(G + 1) * GRP, 0:nb],
                    ps.rearrange("p (u j) -> p u j", j=64)[:, :, 0:nb],
                    AF.Exp)

            yts = {}

            def ln_part(idx, c0, c1):
                bb, s, nb = plan[idx]
                et, e3 = ets[idx]
                if idx not in yts:
                    if nb == 16:
                        yt_new = ypool.tile([128, C * 16], f16, tag="ysm")
                    else:
                        yt_new = ypool.tile([128, C * 64], f16, tag="y")
                    yts[idx] = yt_new
                yt = yts[idx]
                if nb == 64:
                    nc.scalar.activation(yt[:, c0 * 64:c1 * 64],
                                         et[:, c0 * 64:c1 * 64],
                                         AF.Ln, bias=1.0)
                else:
                    y3 = yt[:, c0 * nb:c1 * nb].rearrange(
                        "p (c j) -> p c j", j=nb)
                    nc.scalar.activation(y3[:, :, :], e3[:, c0:c1, 0:nb],
                                         AF.Ln, bias=1.0)

            def do_store(idx):
                bb, s, nb = plan[idx]
                _, yi = tiles.pop(idx)
                ets.pop(idx)
                # last store on the HWDGE path: shorter tail before final drain
                eng = nc.sync if idx == len(plan) - 1 else nc.gpsimd
                yt = yts.pop(idx)
                eng.dma_start(out=yd[nb][yi], in_=yt[:, 0:C * nb])

            def ln_store(idx):
                bb, s, nb = plan[idx]
                if idx == len(plan) - 1:
                    # overlap the final store with the second Ln half so the
                    # end-of-kernel drain waits on a half-size transfer
                    ln_part(idx, 0, C // 2)
                    _, yi = tiles[idx]
                    yt = yts[idx]
                    nc.sync.dma_start(out=yd[nb][yi][:, 0:(C // 2) * nb],
                                      in_=yt[:, 0:(C // 2) * nb])
                    ln_part(idx, C // 2, C)
                    tiles.pop(idx)
                    ets.pop(idx)
                    yts.pop(idx)
                    nc.sync.dma_start(
                        out=yd[nb][yi][:, (C // 2) * nb:C * nb],
                        in_=yt[:, (C // 2) * nb:C * nb])
                else:
                    ln_part(idx, 0, C)
                    do_store(idx)

            ets = {}

            def new_et(idx):
                et = epool.tile([128, C * 64], f16, tag="e")
                ets[idx] = (et, et.rearrange("p (c j) -> p c j", j=64))

            # ---- stage A: chunk-major wavefront over the first n_a passes.
            # Pass pa joins the wavefront at diagonal off[pa], matching its x
            # slab's DMA arrival so no x-gated cell blocks the in-order PE
            # queue; W2 builds are emitted two per diagonal, ahead of all
            # x2-dependent cells.
            off = TUNE["off"][:n_a]
            for idx in range(n_a):
                new_et(idx)
            for d in range(NGRP - 1 + (off[-1] if off else 0) + 1):
                if TUNE["builds_first"] and not (d == 0 and TUNE.get("d0_cells_first")):
                    for gb in (2 * d + 1, 2 * d + 2):
                        if gb < NGRP:
                            build_w2(gb)
                if d == TUNE.get("w1b_at", -1):
                    for gb in TUNE.get("w1_build", ()):
                        build_w1(gb)
                for pa in range(n_a - 1, -1, -1):
                    G = d - off[pa]
                    if not 0 <= G < NGRP:
                        continue
                    conv_group(G, proc_a[pa])
                    nsp = TUNE.get("ln_splits", 1)
                    per = NGRP // nsp
                    if (G + 1) % per == 0:
                        c0 = (G + 1 - per) * GRP
                        ln_part(proc_a[pa], c0, (G + 1) * GRP)
                    if G == NGRP - 1:
                        do_store(proc_a[pa])
                        if n_a + pa < len(plan):
                            issue_load(n_a + pa)
                if (not TUNE["builds_first"]) or (d == 0 and TUNE.get("d0_cells_first")):
                    for gb in (2 * d + 1, 2 * d + 2):
                        if gb < NGRP:
                            build_w2(gb)

            # ---- stage B: pass-major steady state (first loads were issued
            # at the end of stage A as x buffers freed)
            for idx in range(n_a, len(plan)):
                if idx + n_a < len(plan):
                    issue_load(idx + n_a)
                new_et(idx)
                for G in range(NGRP):
                    conv_group(G, idx)
                ln_store(idx)
    nc.finalize()
    return nc


def kernel(x: np.ndarray, kernels: np.ndarray) -> np.ndarray:
    assert x.shape == (B, T, C) and kernels.shape == (K, 1, C)
    ws1g, ws2g = _build_ws(np.asarray(kernels, dtype=np.float32))
    nc = build_nc(ws1g, ws2g)
    in_maps = []
    for i in range(N_CORES):
        xh = _prep_x(np.asarray(x[i * B_LOCAL:(i + 1) * B_LOCAL],
                                dtype=np.float32))
        in_maps.append(xh)
    res = run_bass_kernel_spmd(nc, in_maps, core_ids=list(range(N_CORES)))
    return np.concatenate([_post_y(r) for r in res.results], axis=0)
